# revision 19
# baseline (speedup 1.0000x reference)
"""Trainium2 Bass kernel for nn_Attention_24215025615017.

8-head spatial attention block (1x1-conv QKV projections with folded BatchNorm,
transposed-softmax attention, exact GELU, output 1x1 conv with folded BN).
Data-parallel over batch: B=32 sharded as 4 batches on each of 8 NeuronCores.

Schedule: the ScalarE exp stream (the algorithmic floor: 8.4M exps per batch)
runs back-to-back; the PE consumes the previous head-pair's probabilities
with dense AV accumulation chains while the current pair's dots keep the exp
pipe fed, so the PE never micro-idles into a HAM re-throttle.

Self-contained: hardcodes shapes/sharding; builds + caches one SPMD Bacc graph.
"""

import sys
import numpy as np

if '/opt/trn_rl_repo' not in sys.path:
    sys.path.insert(0, '/opt/trn_rl_repo')
_a = sys.modules.get('antenv')
if _a is not None and '_ro' in getattr(_a, '__file__', ''):
    # purge the read-only copy so antenv resolves to /opt/trn_rl_repo
    for _m in list(sys.modules):
        if _m == 'antenv' or _m.startswith('antenv.'):
            del sys.modules[_m]

import ml_dtypes

EPS = 1e-5
HEADS = 8
DK = 32
DV = 64
B_TOT = 32
N_CORES = 8
B_LOC = B_TOT // N_CORES  # 4 batches per core
C_IN = 256                # input channels
C_V = 512                 # v channels (h*dv)
N = 1024                  # pixels (32*32)
VSTRIDE = DV + 1          # v_aug block: 64 data cols + ones col

_cache = {}


def _build():
    import concourse.bass as bass
    import concourse.tile as tile
    from concourse import bacc, mybir

    f32 = mybir.dt.float32
    bf16 = mybir.dt.bfloat16
    Exp = mybir.ActivationFunctionType.Exp
    Tanh = mybir.ActivationFunctionType.Tanh
    mult = mybir.AluOpType.mult
    add = mybir.AluOpType.add

    nc = bacc.Bacc("TRN2", target_bir_lowering=False, debug=False,
                   num_devices=N_CORES)

    x_ext = nc.declare_dram_parameter("x", [B_LOC, C_IN, N], bf16, isOutput=False)
    wqT_ext = nc.declare_dram_parameter("wqT", [2, 128, 256], bf16, isOutput=False)
    wkT_ext = nc.declare_dram_parameter("wkT", [2, 128, 256], bf16, isOutput=False)
    wvT_ext = nc.declare_dram_parameter("wvT", [2, 128, 512], bf16, isOutput=False)
    woT_ext = nc.declare_dram_parameter("woT", [4, 128, 256], bf16, isOutput=False)
    shq_ext = nc.declare_dram_parameter("shq", [128, 2], f32, isOutput=False)
    shk_ext = nc.declare_dram_parameter("shk", [128, 2], f32, isOutput=False)
    shv_ext = nc.declare_dram_parameter("shv", [1, 512], bf16, isOutput=False)
    ones_ext = nc.declare_dram_parameter("onesr", [1, 128], bf16, isOutput=False)
    bo_ext = nc.declare_dram_parameter("bo", [128, 2], f32, isOutput=False)
    out_ext = nc.declare_dram_parameter("out", [B_LOC, C_IN, N], f32, isOutput=True)

    from contextlib import ExitStack
    with tile.TileContext(nc) as tc, ExitStack() as ctx:
        consts = ctx.enter_context(tc.tile_pool(name="consts", bufs=1))
        vpool = ctx.enter_context(tc.tile_pool(name="vaug", bufs=1))
        xbp = ctx.enter_context(tc.tile_pool(name="xb", bufs=2))
        qkp = ctx.enter_context(tc.tile_pool(name="qk", bufs=2))
        pp = ctx.enter_context(tc.tile_pool(name="pp", bufs=2))
        gp = ctx.enter_context(tc.tile_pool(name="gp", bufs=3))
        lrp = ctx.enter_context(tc.tile_pool(name="lr", bufs=2))
        rbp = ctx.enter_context(tc.tile_pool(name="rb", bufs=2))
        gbfp = ctx.enter_context(tc.tile_pool(name="gbf", bufs=2))
        osp = ctx.enter_context(tc.tile_pool(name="os", bufs=4))
        # PSUM: 3x [128,1024]f32 (6 banks) for dots/proj — 1.5 chunks of
        # dots lookahead so the exp stream never waits on the dots->exp
        # semaphore roundtrip — plus 2x [128,512]f32 (2 banks) rotating
        # between AV accumulation chains and out-conv psum
        ps_pd = ctx.enter_context(tc.tile_pool(name="psd", bufs=3, space="PSUM"))
        ps_q = ctx.enter_context(tc.tile_pool(name="psq", bufs=2, space="PSUM"))

        # ---- load constants ----
        wq_sb = [consts.tile([128, 256], bf16, tag=f"wq{t}", name=f"wq{t}") for t in range(2)]
        wk_sb = [consts.tile([128, 256], bf16, tag=f"wk{t}", name=f"wk{t}") for t in range(2)]
        wv_sb = [consts.tile([128, 512], bf16, tag=f"wv{t}", name=f"wv{t}") for t in range(2)]
        wo_sb = [consts.tile([128, 256], bf16, tag=f"wo{t}", name=f"wo{t}") for t in range(4)]
        shq_sb = consts.tile([128, 2], f32, tag="shq", name="shq")
        shk_sb = consts.tile([128, 2], f32, tag="shk", name="shk")
        shv_sb = consts.tile([1, 512], bf16, tag="shv", name="shv")
        ones_sb = consts.tile([1, 128], bf16, tag="ones", name="ones")
        bo_sb = consts.tile([128, 2], f32, tag="bo", name="bo")
        # constants go through the Sync engine's DMA queue so the x-input
        # DMA (gpsimd queue) is triggered in parallel, not 10us later
        nc.gpsimd.dma_start(out=ones_sb[:], in_=ones_ext.ap()[:])
        nc.gpsimd.dma_start(out=shq_sb[:], in_=shq_ext.ap()[:])
        nc.gpsimd.dma_start(out=shk_sb[:], in_=shk_ext.ap()[:])
        for t in range(2):
            nc.gpsimd.dma_start(out=wq_sb[t][:], in_=wqT_ext.ap()[t])
            nc.gpsimd.dma_start(out=wk_sb[t][:], in_=wkT_ext.ap()[t])
            nc.gpsimd.dma_start(out=wv_sb[t][:], in_=wvT_ext.ap()[t])
        for t in range(4):
            nc.gpsimd.dma_start(out=wo_sb[t][:], in_=woT_ext.ap()[t])
        nc.gpsimd.dma_start(out=shv_sb[:], in_=shv_ext.ap()[:])
        nc.gpsimd.dma_start(out=bo_sb[:], in_=bo_ext.ap()[:])

        # warm the ACT exp/tanh table set immediately (no DMA dependency)
        wtmp = consts.tile([1, 128], bf16, tag="wtmp", name="wtmp")
        wtmp2 = consts.tile([1, 128], bf16, tag="wtmp2", name="wtmp2")
        nc.vector.memset(wtmp[:], 0.25)
        nc.scalar.activation(wtmp2[:], wtmp[:], Exp)

        # two persistent v_aug buffers (ones columns memset once, data columns
        # rewritten per batch; av reads l from the ones column product)
        vaug = [vpool.tile([128, 64 * VSTRIDE], bf16, tag=f"vaug{i}", name=f"vaug{i}") for i in range(2)]
        nc.vector.memset(vaug[0][:], 1.0)
        nc.vector.memset(vaug[1][:], 1.0)

        # V-projection bias as a precomputed [pixel, channel] plane
        bvs = consts.tile([128, 512], bf16, tag="bvs", name="bvs")
        bps = ps_pd.tile([128, 1024], f32, tag="pd", name="pd")
        nc.tensor.matmul(bps[:, 0:512], ones_sb[0:1, 0:128], shv_sb[0:1, :],
                         start=True, stop=True)
        nc.vector.tensor_copy(bvs[:], bps[:, 0:512])

        st = {}  # per-batch tiles: xb, q, k, gbf
        pst = {}  # per-pair tiles: P, g

        def emit_x(b):
            xb = xbp.tile([128, 2048], bf16, tag="xb", name="xb")
            nc.gpsimd.dma_start(
                out=xb[:].rearrange("p (t n) -> p t n", t=2),
                in_=x_ext.ap()[b].rearrange("(t p) n -> p t n", p=128))
            st[b] = {'xb': xb, 'gbf': []}

        def alloc_qk(b):
            for i, key in ((0, 'q'), (1, 'k')):
                st[b][key] = [qkp.tile([128, N], bf16, tag=f"qk{i}{t}",
                                       name=f"qk{i}{t}") for t in range(2)]

        def emit_qk_piece(b, which, t):
            """Project one 128-channel tile of Q (which=0) or K (which=1)."""
            xb = st[b]['xb']
            w_sb, sh_sb = ((wq_sb, shq_sb), (wk_sb, shk_sb))[which]
            dst = st[b]['q' if which == 0 else 'k']
            ps = ps_pd.tile([128, 1024], f32, tag="pd", name="pd")
            for kt in range(2):
                for ih in range(2):
                    nc.tensor.matmul(
                        ps[:, ih * 512:ih * 512 + 512],
                        w_sb[kt][:, t * 128:(t + 1) * 128],
                        xb[:, kt * 1024 + ih * 512:kt * 1024 + ih * 512 + 512],
                        start=(kt == 0), stop=(kt == 1))
            nc.vector.tensor_scalar(
                dst[t][:], ps[:, 0:1024], sh_sb[:, t:t + 1], None, add)

        def emit_v_piece(b, half):
            """V projection, transposed ([pixel, channel]); one 256-pixel half."""
            xb = st[b]['xb']
            va = vaug[b % 2]
            ps = ps_pd.tile([128, 1024], f32, tag="pd", name="pd")
            for cq in range(2):
                jc = half * 2 + cq
                for kt in range(2):
                    nc.tensor.matmul(
                        ps[:, cq * 512:(cq + 1) * 512],
                        xb[:, kt * 1024 + jc * 128:kt * 1024 + jc * 128 + 128],
                        wv_sb[kt][:],
                        start=(kt == 0), stop=(kt == 1))
            for cq in range(2):
                jc = half * 2 + cq
                src = ps[:, cq * 512:(cq + 1) * 512].rearrange(
                    "p (h d) -> p h d", h=8)
                bsrc = bvs[:].rearrange("p (h d) -> p h d", h=8)
                dst = va[:, jc * 8 * VSTRIDE:(jc + 1) * 8 * VSTRIDE].rearrange(
                    "p (h e) -> p h e", h=8)[:, :, 0:DV]
                nc.vector.tensor_tensor(dst, src, bsrc, add)

        def emit_dots_exp_one(g, jc, hi):
            """Dots + exp for pair g, pixel chunk jc, head hi."""
            b, p = divmod(g, 4)
            h0 = 2 * p
            t_q = h0 // 4
            off = 32 * (h0 % 4) + 32 * hi
            q_sb, k_sb = st[b]['q'], st[b]['k']
            P = pst[g]['P']
            pd = ps_pd.tile([128, 1024], f32, tag="pd", name="pd")
            for ih in range(2):
                nc.tensor.matmul(
                    pd[:, ih * 512:ih * 512 + 512],
                    k_sb[t_q][off:off + 32, jc * 128:(jc + 1) * 128],
                    q_sb[t_q][off:off + 32, ih * 512:(ih + 1) * 512],
                    start=True, stop=True,
                    tile_position=(off, 0))
            nc.scalar.activation(
                P[:, hi * 8192 + jc * 1024:hi * 8192 + (jc + 1) * 1024],
                pd[:], Exp)

        def emit_dots_exp(g, jc):
            for hi in (0, 1):
                emit_dots_exp_one(g, jc, hi)

        def alloc_av_head(g, hi):
            # 2 quarter accumulators ([65 x 512], one PSUM bank each); the
            # two chains interleave so PE drain latency stays hidden
            pst[g].setdefault('av', {})
            for ih in (0, 1):
                pst[g]['av'][(hi, ih)] = ps_q.tile([128, 512], f32, tag="q",
                                                   name="avq")

        def emit_av_steps(g, hi, jcs):
            """Contraction steps (pixel chunks jcs) of head hi's two AV
            quarter chains of pair g; the ih pair shares each va block."""
            b, p = divmod(g, 4)
            h = 2 * p + hi
            va = vaug[b % 2]
            P = pst[g]['P']
            av = pst[g]['av']
            for jc in jcs:
                for ih in (0, 1):
                    nc.tensor.matmul(
                        av[(hi, ih)][0:65, :],
                        va[:, jc * 8 * VSTRIDE + h * VSTRIDE:
                           jc * 8 * VSTRIDE + h * VSTRIDE + VSTRIDE],
                        P[:, hi * 8192 + jc * 1024 + ih * 512:
                          hi * 8192 + jc * 1024 + ih * 512 + 512],
                        start=(jc == 0), stop=(jc == 7))

        def emit_norm_quarter(g, hi, ih):
            """Softmax-normalize one av quarter of pair g: l sits in row 64
            (ones-column product); staged through partition 0. Short per-
            quarter chains keep the copy->recip->bcast->mult latency low."""
            av_q = pst[g]['av'][(hi, ih)]
            gt = pst[g]['g']
            lsb = lrp.tile([1, 512], f32, tag="lsb", name="lsb")
            nc.vector.tensor_copy(lsb[0:1, :], av_q[64:65, :])
            rha = lrp.tile([1, 512], f32, tag="rha", name="rha")
            nc.vector.reciprocal_approx_fast(rha[0:1, :], lsb[0:1, :])
            R = rbp.tile([64, 512], f32, tag="R", name="R")
            nc.gpsimd.partition_broadcast(R[:], rha[0:1, :])
            nc.vector.tensor_tensor(
                gt[hi * 64:hi * 64 + 64, ih * 512:(ih + 1) * 512],
                av_q[0:64, :], R[:], mult)

        def emit_gelu_pre(g):
            # gelu via tanh form (tanh shares the exp ACT table set;
            # the 0.5 factor is folded into the output weights):
            # gelu(x)/0.5 = x * (1 + tanh(c*x + c*0.044715*x^3))
            GC = 0.7978845608028654
            GA = GC * 0.044715
            gt = pst[g]['g']
            t1 = lrp.tile([128, N], bf16, tag="t1", name="t1")
            nc.vector.scalar_tensor_tensor(t1[:], gt[:], GA, gt[:], mult, mult)
            t2 = lrp.tile([128, N], bf16, tag="t2", name="t2")
            nc.vector.scalar_tensor_tensor(t2[:], t1[:], GC, gt[:], add, mult)
            pst[g]['t2'] = t2

        def emit_tanh(g):
            t3 = lrp.tile([128, N], bf16, tag="t3", name="t3")
            nc.scalar.activation(t3[:], pst[g]['t2'][:], Tanh)
            pst[g]['t3'] = t3

        def emit_gelu_post(g):
            b, p = divmod(g, 4)
            gt = pst[g]['g']
            gbf = gbfp.tile([128, N], bf16, tag=f"gbf{p}", name=f"gbf{p}")
            st[b]['gbf'].append(gbf)
            nc.vector.scalar_tensor_tensor(gbf[:], pst[g]['t3'][:], 1.0, gt[:],
                                           add, mult)

        def emit_outconv_piece(b, ot, ih):
            """One [128 chan x 512 pixel] quarter of the output 1x1 conv."""
            gbf = st[b]['gbf']
            pso = ps_q.tile([128, 512], f32, tag="q", name="pso")
            for kt in range(4):
                nc.tensor.matmul(
                    pso[:, :],
                    wo_sb[kt][:, ot * 128:(ot + 1) * 128],
                    gbf[kt][:, ih * 512:(ih + 1) * 512],
                    start=(kt == 0), stop=(kt == 3))
            osb = osp.tile([128, 512], f32, tag="osb", name="osb")
            nc.vector.tensor_scalar(osb[:], pso[:, :], bo_sb[:, ot:ot + 1],
                                    None, add)
            nc.gpsimd.dma_start(
                out=out_ext.ap()[b, ot * 128:(ot + 1) * 128,
                                 ih * 512:(ih + 1) * 512],
                in_=osb[:])

        def alloc_pair(g):
            P = pp.tile([128, 2 * 8192], bf16, tag="P", name="P")
            gt = gp.tile([128, N], bf16, tag="g", name="g")
            pst[g] = {'P': P, 'g': gt}

        # ---- emission schedule ----
        # Pair g's dots+exp loop hosts pair g-1's AV chains and softmax-norm
        # at fixed slots (paced so no PE instruction ever waits long on a
        # cross-engine dep: the in-order PE queue head-of-line blocks
        # otherwise). Projections for batch b+1 and the out-conv of batch
        # b-1 ride in a spill queue drained at the free slots.
        extras = []

        def drain(n):
            while n > 0 and extras:
                f, a = extras.pop(0)
                f(*a)
                n -= 1

        def av_part(q, hi, k):
            if k == 0:
                alloc_av_head(q, hi)
            emit_av_steps(q, hi, [2 * k, 2 * k + 1])

        emit_x(0)
        alloc_qk(0)
        emit_qk_piece(0, 0, 0)
        emit_qk_piece(0, 1, 0)
        emit_v_piece(0, 0)
        emit_v_piece(0, 1)
        extras += [(emit_qk_piece, (0, 0, 1)),
                   (emit_qk_piece, (0, 1, 1)),
                   (emit_v_piece, (0, 2)),
                   (emit_v_piece, (0, 3))]

        NPAIR = 4 * B_LOC
        for g in range(NPAIR):
            b, p = divmod(g, 4)
            q = g - 1
            alloc_pair(g)
            if p == 1 and b + 1 < B_LOC:
                # trigger the next batch's input DMA a pair early so the
                # projection pieces never wait on it
                emit_x(b + 1)
                alloc_qk(b + 1)
            if p == 2 and b + 1 < B_LOC:
                for t in (0, 1):
                    extras.append((emit_qk_piece, (b + 1, 0, t)))
                    extras.append((emit_qk_piece, (b + 1, 1, t)))
            if p == 2 and b >= 1:
                for ot in (0, 1):
                    for ih in (0, 1):
                        extras.append((emit_outconv_piece, (b - 1, ot, ih)))
            if p == 3 and b + 1 < B_LOC:
                for half in range(4):
                    extras.append((emit_v_piece, (b + 1, half)))

            if g < NPAIR - 1:
                for jc in range(8):
                    emit_dots_exp(g, jc)
                    if jc == 0:
                        drain(2)
                    elif jc == 1 and g >= 1:
                        av_part(q, 0, 0)
                        av_part(q, 0, 1)
                    elif jc == 2 and g >= 1:
                        av_part(q, 0, 2)
                        av_part(q, 0, 3)
                    elif jc == 3:
                        if g >= 2:
                            emit_tanh(g - 2)
                            emit_gelu_post(g - 2)
                        if g >= 1:
                            emit_norm_quarter(q, 0, 0)
                            emit_norm_quarter(q, 0, 1)
                    elif jc == 4:
                        drain(2)
                    elif jc == 5 and g >= 1:
                        av_part(q, 1, 0)
                        av_part(q, 1, 1)
                    elif jc == 6 and g >= 1:
                        av_part(q, 1, 2)
                        av_part(q, 1, 3)
                    elif jc == 7:
                        if g >= 1:
                            emit_norm_quarter(q, 1, 0)
                            emit_norm_quarter(q, 1, 1)
                        drain(1)
                if g >= 1:
                    emit_gelu_pre(q)
            else:
                # last pair: h0's exps all precede h1's, so h0's AV can
                # overlap h1's exp stream; pair g-1 post-work rides along
                drain(len(extras))
                for u in range(16):
                    hi, jc = divmod(u, 8)
                    emit_dots_exp_one(g, jc, hi)
                    if u == 2:
                        av_part(q, 0, 0)
                        av_part(q, 0, 1)
                    elif u == 3:
                        av_part(q, 0, 2)
                        av_part(q, 0, 3)
                    elif u == 4:
                        emit_tanh(g - 2)
                        emit_gelu_post(g - 2)
                        emit_norm_quarter(q, 0, 0)
                        emit_norm_quarter(q, 0, 1)
                    elif u == 5:
                        av_part(q, 1, 0)
                        av_part(q, 1, 1)
                    elif u == 6:
                        av_part(q, 1, 2)
                        av_part(q, 1, 3)
                    elif u == 7:
                        emit_norm_quarter(q, 1, 0)
                        emit_norm_quarter(q, 1, 1)
                    elif u == 8:
                        emit_gelu_pre(q)
                        av_part(g, 0, 0)
                    elif u == 9:
                        av_part(g, 0, 1)
                    elif u == 10:
                        av_part(g, 0, 2)
                    elif u == 11:
                        av_part(g, 0, 3)
                    elif u == 12:
                        emit_tanh(q)
                        emit_gelu_post(q)
                    elif u == 13:
                        emit_norm_quarter(g, 0, 0)
                        emit_norm_quarter(g, 0, 1)

        # ---- tail: last pair h1's AV/norm/gelu + last batch's out conv ----
        g = NPAIR - 1
        for k in range(4):
            av_part(g, 1, k)
        emit_norm_quarter(g, 1, 0)
        emit_norm_quarter(g, 1, 1)
        emit_gelu_pre(g)
        emit_tanh(g)
        emit_gelu_post(g)
        for ot in (0, 1):
            for ih in (0, 1):
                emit_outconv_piece(B_LOC - 1, ot, ih)

    nc.compile()
    return nc


def _get_nc():
    if 'nc' not in _cache:
        _cache['nc'] = _build()
    return _cache['nc']


def _fold_weights(inputs):
    """Fold BatchNorms (+ attention scale) into conv weights, host-side."""
    f8 = {k: np.asarray(v, np.float64) for k, v in inputs.items()}
    scale = DK ** -0.5

    def fold(w, g, b, m, v, extra=1.0):
        inv = g / np.sqrt(v + EPS)
        return w * inv[:, None] * extra, (b - m * inv) * extra

    wq_e, shq = fold(f8['wq'], f8['gq'], f8['bq'], f8['mq'], f8['vq'], scale)
    wk_e, shk = fold(f8['wk'], f8['gk'], f8['bk'], f8['mk'], f8['vk'])
    wv_e, shv = fold(f8['wv'], f8['gv'], f8['bv'], f8['mv'], f8['vv'])
    inv_o = f8['go'] / np.sqrt(f8['vo'] + EPS)
    # the 0.5 of the tanh-form gelu is folded in here
    wo_e = f8['wo'] * inv_o[:, None] * 0.5
    bo = inv_o * f8['b_out'] + (f8['be_o'] - f8['mo'] * inv_o)

    bf = ml_dtypes.bfloat16
    return {
        'wqT': np.ascontiguousarray(wq_e.T).reshape(2, 128, 256).astype(bf),
        'wkT': np.ascontiguousarray(wk_e.T).reshape(2, 128, 256).astype(bf),
        'wvT': np.ascontiguousarray(wv_e.T).reshape(2, 128, 512).astype(bf),
        'woT': np.ascontiguousarray(wo_e.T).reshape(4, 128, 256).astype(bf),
        'shq': np.ascontiguousarray(shq.reshape(2, 128).T).astype(np.float32),
        'shk': np.ascontiguousarray(shk.reshape(2, 128).T).astype(np.float32),
        'shv': shv.reshape(1, 512).astype(bf),
        'onesr': np.ones((1, 128), bf),
        'bo': np.ascontiguousarray(bo.reshape(2, 128).T).astype(np.float32),
    }


def kernel_run(inputs, trace=False, trace_kwargs=None):
    from concourse.bass_utils import run_bass_kernel_spmd
    nc = _get_nc()
    consts = _fold_weights(inputs)
    x = np.asarray(inputs['x'], np.float32).reshape(B_TOT, C_IN, N)
    x = x.astype(ml_dtypes.bfloat16)
    in_maps = []
    for c in range(N_CORES):
        m = dict(consts)
        m['x'] = np.ascontiguousarray(x[c * B_LOC:(c + 1) * B_LOC])
        in_maps.append(m)
    res = run_bass_kernel_spmd(nc, in_maps, core_ids=list(range(N_CORES)),
                               trace=trace, **(trace_kwargs or {}))
    out = np.concatenate([res.results[c]['out'] for c in range(N_CORES)], axis=0)
    return out.reshape(B_TOT, C_IN, 32, 32), res


def kernel(**inputs) -> np.ndarray:
    out, _ = kernel_run(inputs, trace=False)
    return out
